# revision 17
# baseline (speedup 1.0000x reference)
"""Trainium2 Bass kernel for nn_ButterflyFactorNewMlp.

Computes: attn = einsum('ds,td->st', w1, w2) * sparse_mask
          out  = gelu(einsum('bds,st->bdt', x, attn) + b2)   (exact erf gelu)

Key structural facts (hardcoded):
  - x: [64, 768, 729] f32; w1: [2916, 729]; w2: [729, 2916]; b2: [729]
  - sparse_mask[s,t] != 0  iff  s//81 == t//81 and (s%27)//3 == (t%27)//3.
    Writing s = 81A + 27u + 3k + r, the host permutes both the s and t axes
    by pos = 81A + 9k + 3u + r, which turns attn into a block-diagonal
    matrix of 81 dense 9x9 blocks.  Split into 6 chunks of 126 = 14 blocks
    (chunk 5: 11 blocks / 99 valid), attn is 6 independent 126x126 tiles.
  - Sharding: data-parallel on batch (8 batches / 6144 tokens per core);
    the small attn computation is replicated on every core (collectives
    cost ~100us of serialized startup/barrier, measured in an earlier
    session -- replication wins).

Layouts (host prep is layout/dtype only, all math stays on device):
  - Every large DRAM operand is stored so each per-DMA source region is
    fully CONTIGUOUS and its SBUF side covers all 128 partitions with one
    contiguous run per partition (12KB for x, 6KB for weights/out).
    HWDGE only spreads a transfer across the 16 SDMA engines under those
    conditions; a 127-partition subtile dest measured ~25 GB/s (single
    engine) vs 400+ GB/s spread.
  - x ships PRE-TRANSPOSED per core in 6 token-blocks of 1024: block dg
    has rows (p*6 + c) = feature-position 126c+p of chunk c (p<126);
    p=126 is a ones row that carries the bias through the contraction,
    p=127 pads to 128 partitions.  No PE transposes in stage 2.
  - All PE-stationary operands are 128 columns wide (weight tables padded
    to 768 cols, attn chunks strided at 128) so the compiler's Fast
    Weight Load kicks in (requires NumWeights==128 and non-fp32).
  - One 6-bank PSUM tile serves both stages: stage-1 accumulates attn
    chunk j in bank j (matmul start=True clears has_written per whole
    bank, so concurrent accumulation chains must not share banks);
    stage-2's 256-token groups alternate bank triples 0-2 / 3-5.  A
    single tile means the stage handoff is plain subtile RAW/WAR
    tracking -- a nested tile-pool boundary here measured ~4us of
    all-engine stall.
  - stage 1: for chunk j, attn_j = w1p[:,j]^T @ w2tp[:,j] accumulated
    over 23 kd-chunks, kd-outer so all 6 accumulators chase the weight
    DMA stream; DVE applies the mask; b2 rides in as row 126 of attn_sb
    via a casting DMA.  stage 2: attn chunk j [127, 128] stationary, x^T
    moving, out^T lands in PSUM; one exact-gelu ACT per 256-token group
    reads [126, 6x256] across 3 banks and writes fp16; stores issue from
    the ACT engine's own HWDGE ring (SP-ring stores head-of-line block
    later x loads).

Precision: fp16 operands, fp32 PSUM accumulation, erf-gelu LUT on the
fp32 accumulator, fp16 stores.  End-to-end ~7e-4 relative to scale.
"""

import sys

if "/opt/trn_rl_repo" not in sys.path:
    sys.path.insert(0, "/opt/trn_rl_repo")

import numpy as np

import concourse.bacc as bacc
import concourse.mybir as mybir
import concourse.tile as tile
from concourse.bass import ds
from concourse.bass_utils import run_bass_kernel_spmd

F32 = mybir.dt.float32
F16 = mybir.dt.float16
GELU = mybir.ActivationFunctionType.Gelu

N_CORES = 8
B, D, S = 64, 768, 729
H = 2916
HP = 2944                       # hidden padded to 23*128
N_KD = HP // 128                # 23 contraction chunks for the attn matmuls
KD_BATCH = 4                    # kd chunks per weight DMA (~1.5MB)
CH = 126                        # attn chunk width (14 blocks of 9)
NCH = 6                         # chunks: 5*126 + 99 (+27 pad) = 756
SPAD = NCH * CH                 # 756 padded feature axis (position space)
WPAD = 768                      # weight-table column pad (128-col stationaries)
CP = CH + 1                     # 127 contraction rows per chunk (126 + ones)
M_PER_CORE = (B // N_CORES) * D  # 6144 tokens per core
DG = 1024                       # tokens per x DMA group
NDG = M_PER_CORE // DG          # 6
QG = 256                        # tokens per matmul/gelu group
NQ = DG // QG                   # 4

_COMPILED = None
LAST = None  # BassKernelResults of the most recent kernel() call (for test.py)


def _build():
    nc = bacc.Bacc("TRN2", target_bir_lowering=False, debug=False)

    # xt block dg: rows p*NCH + c -> (chunk c, contraction row p)
    xt_d = nc.dram_tensor("xt", [NDG * 128 * NCH, DG], F16, kind="ExternalInput")
    # weights: [p, kd, f] so each partition's kd-batch is contiguous
    w1_d = nc.dram_tensor("w1p", [128, N_KD, WPAD], F16, kind="ExternalInput")
    w2t_d = nc.dram_tensor("w2tp", [128, N_KD, WPAD], F16, kind="ExternalInput")
    mask_d = nc.dram_tensor("maskp", [128, NCH, CH], F16, kind="ExternalInput")
    b2_d = nc.dram_tensor("b2p", [1, NCH, CH], F32, kind="ExternalInput")
    # out blocks at 512-token granularity so each store fires right after
    # its two gelu groups (short serial tail, early store overlap)
    out_d = nc.dram_tensor(
        "out", [2 * NDG * CH * NCH, DG // 2], F16, kind="ExternalOutput"
    )

    with tile.TileContext(nc) as tc:
        with (
            tc.tile_pool(name="const", bufs=1) as cpool,
            tc.tile_pool(name="xin", bufs=4) as xpool,
            tc.tile_pool(name="oout", bufs=3) as opool,
            tc.tile_pool(name="psum", bufs=1, space="PSUM") as pspool,
        ):
            x_sbs = [None] * NDG

            def load_x(dg):
                t = xpool.tile([128, NCH, DG], F16, tag="xt", name="x_sb")
                nc.sync.dma_start(
                    t[:, :, :],
                    xt_d[ds(dg * 128 * NCH, 128 * NCH), :].rearrange(
                        "(p c) t -> p c t", p=128
                    ),
                )
                x_sbs[dg] = t

            # ---------------- stage 1: replicated attn ----------------
            # weight DMAs go first: everything depends on attn, so the
            # x prefetches must not delay the weight stream
            w1_sb = cpool.tile([128, N_KD, WPAD], F16)
            w2_sb = cpool.tile([128, N_KD, WPAD], F16)
            for kb in range((N_KD + KD_BATCH - 1) // KD_BATCH):
                k0 = kb * KD_BATCH
                kn = min(KD_BATCH, N_KD - k0)
                nc.sync.dma_start(w1_sb[:, ds(k0, kn), :], w1_d[:, ds(k0, kn), :])
                nc.sync.dma_start(w2_sb[:, ds(k0, kn), :], w2t_d[:, ds(k0, kn), :])
            mask_sb = cpool.tile([128, NCH, CH], F16)
            nc.sync.dma_start(mask_sb[:], mask_d[:])

            # attn in SBUF (fp16), chunk stride 128 for FWL-wide stationary
            # reads: rows 0..125 = masked attn, row 126 = bias row (cast
            # f32 -> f16 during DMA, SWDGE); everything else stays 0
            attn_sb = cpool.tile([128, NCH, 128], F16)
            nc.gpsimd.memset(attn_sb[:].bitcast(F32), 0.0)
            nc.gpsimd.dma_start(attn_sb[CH : CH + 1, :, 0:CH], b2_d[:])

            # preload the gelu ACT table set during the weight DMA window
            warm = cpool.tile([1, 8], F32)
            nc.gpsimd.memset(warm[:], 0.0)
            nc.scalar.activation(warm[:], warm[:], GELU)

            load_x(0)
            load_x(1)

            # one 6-bank PSUM tile for both stages, viewed as 12 groups of
            # 1KB: stage-1 accumulator j uses group 2j (start of bank j --
            # matmul start=True clears has_written per whole bank, so
            # concurrent accumulation chains must not share banks);
            # stage-2's two in-flight q-groups use groups 0-5 / 6-11 so
            # every access is a CONTIGUOUS byte interval (the dep tracker
            # coarsens strided accesses to their bounding box, which
            # serializes PE against ACT).
            ps = pspool.tile([128, 2 * NCH, QG], F32, name="ps")

            # kd-outer: all 6 chunk accumulators chase the weight stream
            for kd in range(N_KD):
                for j in range(NCH):
                    nc.tensor.matmul(
                        ps[:, 2 * j, 0:CH],
                        w1_sb[:, kd, ds(CH * j, 128)],
                        w2_sb[:, kd, ds(CH * j, CH)],
                        start=(kd == 0),
                        stop=(kd == N_KD - 1),
                    )
            for j in range(NCH):
                nc.vector.tensor_tensor(
                    attn_sb[0:CH, j, 0:CH],
                    ps[0:CH, 2 * j, 0:CH],
                    mask_sb[0:CH, j, :],
                    mybir.AluOpType.mult,
                )

            # ---------------- stage 2: main matmul ----------------
            for dg in range(NDG):
                if dg + 2 < NDG:
                    load_x(dg + 2)
                x_sb = x_sbs[dg]
                for half in range(2):
                    o_sb = opool.tile(
                        [128, NCH, DG // 2], F16, tag="o", name="o_sb"
                    )
                    for qq in range(2):
                        q = half * 2 + qq
                        pg = NCH * (q % 2)  # psum group base for this parity
                        for j in range(NCH):
                            nc.tensor.matmul(
                                ps[:, pg + j, :],
                                attn_sb[0:CP, j, :],
                                x_sb[0:CP, j, ds(q * QG, QG)],
                                start=True,
                                stop=True,
                            )
                        nc.scalar.activation(
                            o_sb[0:CH, :, ds(qq * QG, QG)],
                            ps[0:CH, ds(pg, NCH), :],
                            GELU,
                        )
                    # store via the ACT engine's HWDGE ring: it follows
                    # the two gelu ACTs in that engine's own queue, so it
                    # never head-of-line blocks xt loads on the SP ring
                    hg = dg * 2 + half
                    nc.scalar.dma_start(
                        out_d[ds(hg * CH * NCH, CH * NCH), :].rearrange(
                            "(p c) t -> p c t", p=CH
                        ),
                        o_sb[0:CH, :, :],
                    )
                x_sbs[dg] = None

    nc.compile()
    return nc


def _perm():
    s = np.arange(S)
    pos = 81 * (s // 81) + 9 * ((s % 27) // 3) + 3 * ((s % 81) // 27) + (s % 3)
    P = np.empty(S, np.int64)
    P[pos] = s  # P[c] = original feature index at permuted position c
    return P


def _host_prep(w1, w2, b2, mask, P):
    """Permuted fp16 weight/mask/bias tables (layout + dtype only)."""
    w1p = np.zeros((HP, WPAD), np.float16)
    w1p[:H, :S] = w1[:, P]
    w2tp = np.zeros((HP, WPAD), np.float16)
    w2tp[:H, :S] = w2[P, :].T
    # [p, kd, f] layout so each partition's kd-batch rows are contiguous
    w1pd = np.ascontiguousarray(w1p.reshape(N_KD, 128, WPAD).transpose(1, 0, 2))
    w2tpd = np.ascontiguousarray(w2tp.reshape(N_KD, 128, WPAD).transpose(1, 0, 2))
    mp = np.zeros((SPAD, SPAD), np.float32)
    mp[:S, :S] = mask[np.ix_(P, P)]
    maskp = np.zeros((128, NCH, CH), np.float16)
    for j in range(NCH):
        maskp[0:CH, j, :] = mp[j * CH : (j + 1) * CH, j * CH : (j + 1) * CH]
    b2pad = np.zeros(SPAD, np.float32)
    b2pad[:S] = b2[P]
    b2p = np.ascontiguousarray(b2pad.reshape(1, NCH, CH))
    return w1pd, w2tpd, maskp, b2p


def kernel(x, w1, w2, b2, sparse_mask):
    global _COMPILED, LAST
    if _COMPILED is None:
        _COMPILED = _build()
    nc = _COMPILED

    x = np.asarray(x, dtype=np.float32)
    w1 = np.asarray(w1, dtype=np.float32)
    w2 = np.asarray(w2, dtype=np.float32)
    b2 = np.asarray(b2, dtype=np.float32)
    mask = np.asarray(sparse_mask, dtype=np.float32)

    P = _perm()
    w1pd, w2tpd, maskp, b2p = _host_prep(w1, w2, b2, mask, P)

    # x^T in permuted chunk layout: [chunk c, row p, tokens]; p<126 ->
    # feature position 126c+p, p=126 -> ones row, p=127 -> pad
    xf = x.reshape(B * D, S)
    xp_pad = np.zeros((SPAD, B * D), np.float16)
    xp_pad[:S] = xf[:, P].T.astype(np.float16)
    xpT = np.zeros((NCH, 128, B * D), np.float16)
    xpT[:, 0:CH, :] = xp_pad.reshape(NCH, CH, B * D)
    xpT[:, CH, :] = 1.0

    in_maps = []
    for c in range(N_CORES):
        xc = xpT[:, :, c * M_PER_CORE : (c + 1) * M_PER_CORE]
        # -> [dg, p, chunk, t] so each DMA block is contiguous and each
        # partition's bytes are one 12KB run
        xt = np.ascontiguousarray(
            xc.reshape(NCH, 128, NDG, DG).transpose(2, 1, 0, 3)
        ).reshape(NDG * 128 * NCH, DG)
        in_maps.append(
            {
                "xt": xt,
                "w1p": w1pd,
                "w2tp": w2tpd,
                "maskp": maskp,
                "b2p": b2p,
            }
        )

    LAST = run_bass_kernel_spmd(nc, in_maps, list(range(N_CORES)))
    # results: out blocks [hg, p, chunk, t]; position = chunk*126 + p
    outp = np.concatenate(
        [
            LAST.results[c]["out"]
            .reshape(2 * NDG, CH, NCH, DG // 2)
            .transpose(2, 1, 0, 3)
            .reshape(SPAD, M_PER_CORE)
            for c in range(N_CORES)
        ],
        axis=1,
    )  # [756, B*D]
    out = np.empty((B * D, S), np.float32)
    out[:, P] = outp[:S].T.astype(np.float32)
    return out.reshape(B, D, S)


# revision 21
# speedup vs baseline: 1.2105x; 1.2105x over previous
"""Trainium2 Bass kernel for nn_ButterflyFactorNewMlp.

Computes: attn = einsum('ds,td->st', w1, w2) * sparse_mask
          out  = gelu(einsum('bds,st->bdt', x, attn) + b2)   (exact erf gelu)

Key structural facts (hardcoded):
  - x: [64, 768, 729] f32; w1: [2916, 729]; w2: [729, 2916]; b2: [729]
  - sparse_mask[s,t] != 0  iff  s//81 == t//81 and (s%27)//3 == (t%27)//3.
    Writing s = 81A + 27u + 3k + r, the host permutes both the s and t axes
    by pos = 81A + 9k + 3u + r, which turns attn into a block-diagonal
    matrix of 81 dense 9x9 blocks.  Split into 6 chunks of 126 = 14 blocks
    (chunk 5: 11 blocks / 99 valid), attn is 6 independent 126x126 tiles.
  - Sharding: data-parallel on batch (8 batches / 6144 tokens per core);
    the small attn computation is replicated on every core (collectives
    cost ~100us of serialized startup/barrier, measured in an earlier
    session -- replication wins).

Layouts (host prep is layout/dtype only, all math stays on device):
  - Every large DRAM operand is stored so each per-DMA source region is
    fully CONTIGUOUS and its SBUF side covers all 128 partitions with one
    contiguous run per partition (12KB for x, 6KB for weights/out).
    HWDGE only spreads a transfer across the 16 SDMA engines under those
    conditions; a 127-partition subtile dest measured ~25 GB/s (single
    engine) vs 400+ GB/s spread.
  - x ships PRE-TRANSPOSED per core in 6 token-blocks of 1024: block dg
    has rows (p*6 + c) = feature-position 126c+p of chunk c (p<126);
    p=126 is a ones row that carries the bias through the contraction,
    p=127 pads to 128 partitions.  No PE transposes in stage 2.
  - All PE-stationary operands are 128 columns wide (weight tables padded
    to 768 cols, attn chunks strided at 128) so the compiler's Fast
    Weight Load kicks in (requires NumWeights==128 and non-fp32).
  - One 6-bank PSUM tile serves both stages: stage-1 accumulates attn
    chunk j in bank j (matmul start=True clears has_written per whole
    bank, so concurrent accumulation chains must not share banks);
    stage-2's 256-token groups alternate bank triples 0-2 / 3-5.  A
    single tile means the stage handoff is plain subtile RAW/WAR
    tracking -- a nested tile-pool boundary here measured ~4us of
    all-engine stall.
  - stage 1: for chunk j, attn_j = w1p[:,j]^T @ w2tp[:,j] accumulated
    over 23 kd-chunks, kd-outer so all 6 accumulators chase the weight
    DMA stream; DVE applies the mask; b2 rides in as row 126 of attn_sb
    via a casting DMA.  stage 2: attn chunk j [127, 128] stationary, x^T
    moving, out^T lands in PSUM; one exact-gelu ACT per 256-token group
    reads [126, 6x256] across 3 banks and writes fp16; stores issue from
    the ACT engine's own HWDGE ring (SP-ring stores head-of-line block
    later x loads).

Precision: fp16 operands, fp32 PSUM accumulation, erf-gelu LUT on the
fp32 accumulator, fp16 stores.  End-to-end ~7e-4 relative to scale.
"""

import sys

if "/opt/trn_rl_repo" not in sys.path:
    sys.path.insert(0, "/opt/trn_rl_repo")

import numpy as np

import concourse.bacc as bacc
import concourse.mybir as mybir
import concourse.tile as tile
from concourse.bass import ds
from concourse.bass_utils import run_bass_kernel_spmd

F32 = mybir.dt.float32
F16 = mybir.dt.float16
GELU = mybir.ActivationFunctionType.Gelu

N_CORES = 8
B, D, S = 64, 768, 729
H = 2916
HP = 2944                       # hidden padded to 23*128
N_KD = HP // 128                # 23 contraction chunks for the attn matmuls
KD_BATCH = 4                    # kd chunks per weight DMA (~1.5MB)
CH = 126                        # attn chunk width (14 blocks of 9)
NCH = 6                         # chunks: 5*126 + 99 (+27 pad) = 756
SPAD = NCH * CH                 # 756 padded feature axis (position space)
WPAD = 768                      # weight-table column pad (128-col stationaries)
CP = CH + 1                     # 127 contraction rows per chunk (126 + ones)
M_PER_CORE = (B // N_CORES) * D  # 6144 tokens per core
DG = 1024                       # tokens per x DMA group
NDG = M_PER_CORE // DG          # 6
QG = 256                        # tokens per matmul/gelu group
NQ = DG // QG                   # 4

_COMPILED = None
LAST = None  # BassKernelResults of the most recent kernel() call (for test.py)


def _build():
    nc = bacc.Bacc("TRN2", target_bir_lowering=False, debug=False)

    # xt block dg: rows p*NCH + c -> (chunk c, contraction row p)
    xt_d = nc.dram_tensor("xt", [NDG * 128 * NCH, DG], F16, kind="ExternalInput")
    # weights: [p, kd, f] so each partition's kd-batch is contiguous
    w1_d = nc.dram_tensor("w1p", [128, N_KD, WPAD], F16, kind="ExternalInput")
    w2t_d = nc.dram_tensor("w2tp", [128, N_KD, WPAD], F16, kind="ExternalInput")
    mask_d = nc.dram_tensor("maskp", [128, NCH, CH], F16, kind="ExternalInput")
    b2_d = nc.dram_tensor("b2p", [1, NCH, CH], F32, kind="ExternalInput")
    # out blocks at 512-token granularity so each store fires right after
    # its two gelu groups (short serial tail, early store overlap)
    out_d = nc.dram_tensor(
        "out", [2 * NDG * CH * NCH, DG // 2], F16, kind="ExternalOutput"
    )

    with tile.TileContext(nc) as tc:
        with (
            tc.tile_pool(name="const", bufs=1) as cpool,
            tc.tile_pool(name="xin", bufs=4) as xpool,
            tc.tile_pool(name="oout", bufs=3) as opool,
            tc.tile_pool(name="psum", bufs=2, space="PSUM") as pspool,
        ):
            x_sbs = [None] * NDG

            def load_x(dg):
                t = xpool.tile([128, NCH, DG], F16, tag="xt", name="x_sb")
                nc.sync.dma_start(
                    t[:, :, :],
                    xt_d[ds(dg * 128 * NCH, 128 * NCH), :].rearrange(
                        "(p c) t -> p c t", p=128
                    ),
                )
                x_sbs[dg] = t

            # ---------------- stage 1: replicated attn ----------------
            # weight DMAs go first: everything depends on attn, so the
            # x prefetches must not delay the weight stream
            w1_sb = cpool.tile([128, N_KD, WPAD], F16)
            w2_sb = cpool.tile([128, N_KD, WPAD], F16)
            for kb in range((N_KD + KD_BATCH - 1) // KD_BATCH):
                k0 = kb * KD_BATCH
                kn = min(KD_BATCH, N_KD - k0)
                nc.sync.dma_start(w1_sb[:, ds(k0, kn), :], w1_d[:, ds(k0, kn), :])
                nc.sync.dma_start(w2_sb[:, ds(k0, kn), :], w2t_d[:, ds(k0, kn), :])
            mask_sb = cpool.tile([128, NCH, CH], F16)
            nc.sync.dma_start(mask_sb[:], mask_d[:])

            # attn in SBUF (fp16), chunk stride 128 for FWL-wide stationary
            # reads: rows 0..125 = masked attn, row 126 = bias row (cast
            # f32 -> f16 during DMA, SWDGE); everything else stays 0
            attn_sb = cpool.tile([128, NCH, 128], F16)
            nc.gpsimd.memset(attn_sb[:].bitcast(F32), 0.0)
            nc.gpsimd.dma_start(attn_sb[CH : CH + 1, :, 0:CH], b2_d[:])

            # preload the gelu ACT table set during the weight DMA window
            warm = cpool.tile([1, 8], F32)
            nc.gpsimd.memset(warm[:], 0.0)
            nc.scalar.activation(warm[:], warm[:], GELU)

            load_x(0)
            load_x(1)

            # stage-1 accumulators come from the SAME pool ring stage-2
            # uses (two 3-bank tiles; chunk accumulator j at group 2*(j%3)
            # = start of its own bank -- matmul start=True clears
            # has_written per whole bank, so concurrent accumulation
            # chains must not share banks).  Same-ring reuse gives the
            # stage handoff for free; PSUM deps between ops on one tile
            # are effectively tile-granular, so distinct tiles per
            # q-group are required for PE/ACT pipelining, and a nested
            # pool boundary here measured ~4us of all-engine stall.
            s1ps = [
                pspool.tile([128, NCH, QG], F32, tag="ps", name=f"s1ps_{h}")
                for h in range(2)
            ]

            # kd-outer: all 6 chunk accumulators chase the weight stream
            for kd in range(N_KD):
                for j in range(NCH):
                    nc.tensor.matmul(
                        s1ps[j // 3][:, 2 * (j % 3), 0:CH],
                        w1_sb[:, kd, ds(CH * j, 128)],
                        w2_sb[:, kd, ds(CH * j, CH)],
                        start=(kd == 0),
                        stop=(kd == N_KD - 1),
                    )
            for j in range(NCH):
                nc.vector.tensor_tensor(
                    attn_sb[0:CH, j, 0:CH],
                    s1ps[j // 3][0:CH, 2 * (j % 3), 0:CH],
                    mask_sb[0:CH, j, :],
                    mybir.AluOpType.mult,
                )

            # ---------------- stage 2: main matmul ----------------
            for dg in range(NDG):
                if dg + 2 < NDG:
                    load_x(dg + 2)
                x_sb = x_sbs[dg]
                for half in range(2):
                    o_sb = opool.tile(
                        [128, NCH, DG // 2], F16, tag="o", name="o_sb"
                    )
                    for qq in range(2):
                        q = half * 2 + qq
                        ps2 = pspool.tile(
                            [128, NCH, QG], F32, tag="ps", name="ps2"
                        )
                        for j in range(NCH):
                            nc.tensor.matmul(
                                ps2[:, j, :],
                                attn_sb[0:CP, j, :],
                                x_sb[0:CP, j, ds(q * QG, QG)],
                                start=True,
                                stop=True,
                            )
                        nc.scalar.activation(
                            o_sb[0:CH, :, ds(qq * QG, QG)],
                            ps2[0:CH, :, :],
                            GELU,
                        )
                    # store via the ACT engine's HWDGE ring: it follows
                    # the two gelu ACTs in that engine's own queue, so it
                    # never head-of-line blocks xt loads on the SP ring
                    hg = dg * 2 + half
                    nc.scalar.dma_start(
                        out_d[ds(hg * CH * NCH, CH * NCH), :].rearrange(
                            "(p c) t -> p c t", p=CH
                        ),
                        o_sb[0:CH, :, :],
                    )
                x_sbs[dg] = None

    nc.compile()
    return nc


def _perm():
    s = np.arange(S)
    pos = 81 * (s // 81) + 9 * ((s % 27) // 3) + 3 * ((s % 81) // 27) + (s % 3)
    P = np.empty(S, np.int64)
    P[pos] = s  # P[c] = original feature index at permuted position c
    return P


def _host_prep(w1, w2, b2, mask, P):
    """Permuted fp16 weight/mask/bias tables (layout + dtype only)."""
    w1p = np.zeros((HP, WPAD), np.float16)
    w1p[:H, :S] = w1[:, P]
    w2tp = np.zeros((HP, WPAD), np.float16)
    w2tp[:H, :S] = w2[P, :].T
    # [p, kd, f] layout so each partition's kd-batch rows are contiguous
    w1pd = np.ascontiguousarray(w1p.reshape(N_KD, 128, WPAD).transpose(1, 0, 2))
    w2tpd = np.ascontiguousarray(w2tp.reshape(N_KD, 128, WPAD).transpose(1, 0, 2))
    mp = np.zeros((SPAD, SPAD), np.float32)
    mp[:S, :S] = mask[np.ix_(P, P)]
    maskp = np.zeros((128, NCH, CH), np.float16)
    for j in range(NCH):
        maskp[0:CH, j, :] = mp[j * CH : (j + 1) * CH, j * CH : (j + 1) * CH]
    b2pad = np.zeros(SPAD, np.float32)
    b2pad[:S] = b2[P]
    b2p = np.ascontiguousarray(b2pad.reshape(1, NCH, CH))
    return w1pd, w2tpd, maskp, b2p


def kernel(x, w1, w2, b2, sparse_mask):
    global _COMPILED, LAST
    if _COMPILED is None:
        _COMPILED = _build()
    nc = _COMPILED

    x = np.asarray(x, dtype=np.float32)
    w1 = np.asarray(w1, dtype=np.float32)
    w2 = np.asarray(w2, dtype=np.float32)
    b2 = np.asarray(b2, dtype=np.float32)
    mask = np.asarray(sparse_mask, dtype=np.float32)

    P = _perm()
    w1pd, w2tpd, maskp, b2p = _host_prep(w1, w2, b2, mask, P)

    # x^T in permuted chunk layout: [chunk c, row p, tokens]; p<126 ->
    # feature position 126c+p, p=126 -> ones row, p=127 -> pad
    xf = x.reshape(B * D, S)
    xp_pad = np.zeros((SPAD, B * D), np.float16)
    xp_pad[:S] = xf[:, P].T.astype(np.float16)
    xpT = np.zeros((NCH, 128, B * D), np.float16)
    xpT[:, 0:CH, :] = xp_pad.reshape(NCH, CH, B * D)
    xpT[:, CH, :] = 1.0

    in_maps = []
    for c in range(N_CORES):
        xc = xpT[:, :, c * M_PER_CORE : (c + 1) * M_PER_CORE]
        # -> [dg, p, chunk, t] so each DMA block is contiguous and each
        # partition's bytes are one 12KB run
        xt = np.ascontiguousarray(
            xc.reshape(NCH, 128, NDG, DG).transpose(2, 1, 0, 3)
        ).reshape(NDG * 128 * NCH, DG)
        in_maps.append(
            {
                "xt": xt,
                "w1p": w1pd,
                "w2tp": w2tpd,
                "maskp": maskp,
                "b2p": b2p,
            }
        )

    LAST = run_bass_kernel_spmd(nc, in_maps, list(range(N_CORES)))
    # results: out blocks [hg, p, chunk, t]; position = chunk*126 + p
    outp = np.concatenate(
        [
            LAST.results[c]["out"]
            .reshape(2 * NDG, CH, NCH, DG // 2)
            .transpose(2, 1, 0, 3)
            .reshape(SPAD, M_PER_CORE)
            for c in range(N_CORES)
        ],
        axis=1,
    )  # [756, B*D]
    out = np.empty((B * D, S), np.float32)
    out[:, P] = outp[:S].T.astype(np.float32)
    return out.reshape(B, D, S)
